# revision 1
# baseline (speedup 1.0000x reference)
"""TRN2 Bass kernel for nn_GCNEModel (3-layer GCN + dense head), 8 NeuronCores.

Sharding: data-parallel over the batch axis — each core runs one sample's
full GCN. The scatter-add aggregation is restructured as pure row-gathers:
nodes are relabeled by descending in-degree (the relabeling is absorbed into
host-side permutations of pel_W / x / lin1_W), so "round k" (the k-th
incoming edge of every node that has one) targets a contiguous node prefix.
Per layer, per core:

  t = x @ W^T            PE GEMM (stationary = feat-major input chunk)
  m = dinv * t           DVE, node-major [128, 119, 64]; DMA'd to HBM rows
  s = m + sum_k m[src_k] MoE-style dma_gather rounds + DVE adds
  h = relu(dinv*s^T + b) DVE mult, PE transpose, ACT bias+relu (feat-major)
  g += h^T @ w_l         PE matvec chunks (fc folded per layer)

head: z = relu(sum_j g_j^T @ lin1_W^T_j + b1); host: lin2 + log_softmax.
"""
import os
import sys

os.environ.setdefault("NEURON_RT_RESET_CORES", "1")
for _p in ("/opt/trn_rl_repo", "/root/.axon_site/_ro/trn_rl_repo"):
    if os.path.isdir(_p) and _p not in sys.path:
        sys.path.insert(0, _p)

from dataclasses import dataclass, field

import numpy as np

import concourse.bacc as bacc
import concourse.mybir as mybir
import concourse.tile as tile
from concourse.bass_utils import run_bass_kernel_spmd

P = 128
HID = 64
FIN = 36
NFC = 256
N_CORES = 8


@dataclass
class Cfg:
    n: int
    chunk: int = 3968   # gather chunk; <= ~4080 (SWDGE ring: 256 descs/engine)
    npad: int = field(init=False)
    nslot: int = field(init=False)

    def __post_init__(self):
        self.nslot = (self.n + P - 1) // P
        self.npad = self.nslot * P


def preprocess(cfg: Cfg, edge_index: np.ndarray):
    n, npad, nslot = cfg.n, cfg.npad, cfg.nslot
    src_old = np.asarray(edge_index[0], dtype=np.int64)
    dst_old = np.asarray(edge_index[1], dtype=np.int64)
    E = src_old.shape[0]

    deg = np.bincount(dst_old, minlength=n)
    pi = np.argsort(-deg, kind="stable")
    inv_pi = np.empty(n, dtype=np.int64)
    inv_pi[pi] = np.arange(n)
    deg_s = deg[pi]

    dinv_pad = np.zeros(npad)
    dinv_pad[:n] = 1.0 / np.sqrt(deg_s.astype(np.float64) + 1.0)

    src_new = inv_pi[src_old]
    dst_new = inv_pi[dst_old]

    order = np.argsort(dst_new, kind="stable")
    src_sorted = src_new[order]
    dst_sorted = dst_new[order]
    starts = np.zeros(n + 1, dtype=np.int64)
    np.cumsum(np.bincount(dst_new, minlength=n), out=starts[1:])
    kpos = np.arange(E) - starts[dst_sorted]

    Kmax = int(deg_s[0]) if E else 0
    DUMMY_SIG = n
    assert npad > n

    def r_of(sig):
        return (sig % P) * nslot + sig // P

    idx_stream = []
    segments = []
    stream_blk = 0
    for k in range(Kmax):
        n_k = int(np.searchsorted(-deg_s, -(k + 1), side="right"))
        sel = kpos == k
        srcs_k = src_sorted[sel][np.argsort(dst_sorted[sel], kind="stable")]
        assert srcs_k.shape[0] == n_k
        nblk = (n_k + P - 1) // P
        padded = np.full(nblk * P, DUMMY_SIG, dtype=np.int64)
        padded[:n_k] = srcs_k
        idx_stream.append(r_of(padded))
        segments.append((stream_blk, nblk))
        stream_blk += nblk

    idx_stream = (np.concatenate(idx_stream) if idx_stream
                  else np.zeros(0, np.int64))
    Eprime = idx_stream.shape[0]

    chunks = []
    pos = 0
    while pos < Eprime:
        c = min(cfg.chunk, Eprime - pos)
        chunks.append((pos, c))
        pos += c

    chunk_adds = []
    for (cstart, clen) in chunks:
        c_b0, c_b1 = cstart // P, (cstart + clen) // P
        adds = []
        for (sb, nb) in segments:
            lo, hi = max(c_b0, sb), min(c_b1, sb + nb)
            if lo < hi:
                adds.append((lo - c_b0, lo - sb, hi - lo))
        chunk_adds.append(adds)

    cols = Eprime // 16
    arr16 = idx_stream.reshape(cols, 16).T.astype(np.int16)
    idx_wrapped = np.ascontiguousarray(np.tile(arr16, (8, 1)))

    return dict(pi=pi, deg_s=deg_s, dinv_pad=dinv_pad,
                idx_stream=idx_stream, idx_wrapped=idx_wrapped,
                chunks=chunks, chunk_adds=chunk_adds, Eprime=Eprime)


def build_constants(cfg: Cfg, prep, inputs):
    n, npad, nslot = cfg.n, cfg.npad, cfg.nslot
    pi = prep["pi"]
    dinv_pad = prep["dinv_pad"].astype(np.float32)

    pel_W = np.asarray(inputs["pel_W"], np.float32)
    pel_b = np.asarray(inputs["pel_b"], np.float32)
    pe_perm = (pel_W.T + pel_b)[pi]

    x = np.asarray(inputs["x"], np.float32)
    bs = x.shape[0]
    x_fm = np.zeros((bs, FIN, npad), np.float32)
    for s in range(bs):
        xc = np.concatenate([x[s][pi], pe_perm], axis=1)
        x_fm[s, :, :n] = xc.T

    def to_node_major(v):
        return np.ascontiguousarray(v.reshape(nslot, P).T)

    dinv_nm = to_node_major(dinv_pad)
    dinv64 = np.ascontiguousarray(
        np.repeat(dinv_nm[:, :, None], HID, axis=2)).reshape(P, nslot * HID)
    mask = np.zeros(npad, np.float32)
    mask[:n] = 1.0
    mask_nm = to_node_major(mask)

    Wc = [np.ascontiguousarray(np.asarray(inputs[f"conv{i}_W"], np.float32).T)
          for i in (1, 2, 3)]
    bc = [np.ascontiguousarray(np.asarray(inputs[f"conv{i}_b"], np.float32)
                               .reshape(HID, 1)) for i in (1, 2, 3)]

    fc_W = np.asarray(inputs["fc_W"], np.float32).reshape(-1)
    w_l = [np.ascontiguousarray(fc_W[l::3].reshape(HID, 1)) for l in range(3)]
    fc_b = float(np.asarray(inputs["fc_b"], np.float32).reshape(()))

    lin1_W = np.asarray(inputs["lin1_W"], np.float32)
    W1T = np.zeros((npad, NFC), np.float32)
    W1T[:n] = lin1_W[:, pi].T
    b1_eff = np.ascontiguousarray(
        (np.asarray(inputs["lin1_b"], np.float32)
         + fc_b * lin1_W.sum(axis=1)).reshape(1, NFC))

    return dict(x_fm=x_fm, dinv64=dinv64, mask_nm=mask_nm, Wc=Wc, bc=bc,
                w_l=w_l, W1T=W1T, b1_eff=b1_eff)


def build_program(cfg: Cfg, prep, gb_bufs=2, idx_bufs=4, xw_bufs=2, w1_bufs=4, pt_bufs=3, ptr_bufs=2, gemm_grp=8, tr_grp=4):
    n, npad, nslot = cfg.n, cfg.npad, cfg.nslot
    chunks, chunk_adds = prep["chunks"], prep["chunk_adds"]
    cols_total = prep["Eprime"] // 16
    f32 = mybir.dt.float32

    nc = bacc.Bacc("TRN2", debug=False)

    x_dram = nc.dram_tensor("x_fm", [FIN, npad], f32, kind="ExternalInput")
    dinv_dram = nc.dram_tensor("dinv64", [P, nslot * HID], f32, kind="ExternalInput")
    mask_dram = nc.dram_tensor("mask_nm", [P, nslot], f32, kind="ExternalInput")
    Wc_dram = [nc.dram_tensor(f"Wc{i}", [FIN if i == 0 else HID, HID], f32,
                              kind="ExternalInput") for i in range(3)]
    bc_dram = [nc.dram_tensor(f"bc{i}", [HID, 1], f32, kind="ExternalInput")
               for i in range(3)]
    wl_dram = [nc.dram_tensor(f"wl{i}", [HID, 1], f32, kind="ExternalInput")
               for i in range(3)]
    idx_dram = nc.dram_tensor("idx_all", [P, cols_total], mybir.dt.int16,
                              kind="ExternalInput")
    w1t_dram = nc.dram_tensor("W1T", [npad, NFC], f32, kind="ExternalInput")
    b1_dram = nc.dram_tensor("b1_eff", [1, NFC], f32, kind="ExternalInput")
    ident_dram = nc.dram_tensor("ident", [P, P], f32, kind="ExternalInput")
    z_dram = nc.dram_tensor("z", [1, NFC], f32, kind="ExternalOutput")

    m_dram = [nc.dram_tensor(f"m_hbm{i}", [npad, HID], f32) for i in range(3)]

    with tile.TileContext(nc) as tc:
        with (
            tc.tile_pool(name="const", bufs=1) as cpool,
            tc.tile_pool(name="state", bufs=1) as spool,
            tc.tile_pool(name="xw", bufs=xw_bufs) as xpool,
            tc.tile_pool(name="idx", bufs=idx_bufs) as ipool,
            tc.tile_pool(name="gath", bufs=gb_bufs) as gpool,
            tc.tile_pool(name="w1t", bufs=w1_bufs) as wpool,
            tc.tile_pool(name="psum_t", bufs=pt_bufs, space="PSUM") as pt_pool,
            tc.tile_pool(name="psum_tr", bufs=ptr_bufs, space="PSUM") as ptr_pool,
            tc.tile_pool(name="psum_g", bufs=1, space="PSUM") as pg_pool,
            tc.tile_pool(name="psum_z", bufs=1, space="PSUM") as pz_pool,
        ):
            dinv64 = cpool.tile([P, nslot, HID], f32, tag="dinv64")
            nc.sync.dma_start(out=dinv64[:], in_=dinv_dram[:].rearrange(
                "p (g f) -> p g f", f=HID))
            mask_sb = cpool.tile([P, nslot], f32, tag="mask")
            nc.sync.dma_start(out=mask_sb[:], in_=mask_dram[:])
            ident = cpool.tile([P, P], f32, tag="ident")
            nc.sync.dma_start(out=ident[:], in_=ident_dram[:])
            Wc_sb, bc_sb, wl_sb = [], [], []
            for i in range(3):
                w = cpool.tile([FIN if i == 0 else HID, HID], f32, tag=f"Wc{i}")
                nc.sync.dma_start(out=w[:], in_=Wc_dram[i][:])
                Wc_sb.append(w)
                b = cpool.tile([HID, 1], f32, tag=f"bc{i}")
                nc.sync.dma_start(out=b[:], in_=bc_dram[i][:])
                bc_sb.append(b)
                wl = cpool.tile([HID, 1], f32, tag=f"wl{i}")
                nc.sync.dma_start(out=wl[:], in_=wl_dram[i][:])
                wl_sb.append(wl)
            b1_sb = cpool.tile([1, NFC], f32, tag="b1")
            nc.sync.dma_start(out=b1_sb[:], in_=b1_dram[:])

            g_acc = spool.tile([P, nslot], f32, tag="g_acc")
            nc.vector.memset(g_acc[:], 0.0)
            m_sb = spool.tile([P, nslot, HID], f32, tag="m")
            s_sb = spool.tile([P, nslot, HID], f32, tag="s")
            h_fm = spool.tile([HID, npad], f32, tag="h_fm")

            GEMM_GRP = gemm_grp
            TR_GRP = tr_grp
            HGRP = 4
            psum_z = pz_pool.tile([1, NFC], f32, tag="pz")
            head_state = dict(emitted=0, total=nslot)

            def emit_head_seg(si, b0, b1):
                # finalize g for this segment and fold it into the lin1
                # matvec while layer-3 gathers still run
                nc.vector.tensor_add(g_acc[:, b0:b1], g_acc[:, b0:b1],
                                     psum_g[:, b0:b1])
                nc.vector.tensor_mul(g_acc[:, b0:b1], g_acc[:, b0:b1],
                                     mask_sb[:, b0:b1])
                for g0 in range(b0, b1, HGRP):
                    gn = min(HGRP, b1 - g0)
                    w1t = wpool.tile([P, HGRP, NFC], f32, tag="w1t")
                    nc.sync.dma_start(
                        out=w1t[:, :gn, :],
                        in_=w1t_dram[:].rearrange(
                            "(g p) f -> p g f", p=P)[:, g0:g0 + gn, :])
                    for j in range(gn):
                        jj = g0 + j
                        nc.tensor.matmul(
                            psum_z[:], g_acc[:, jj:jj + 1], w1t[:, j, :],
                            start=(head_state["emitted"] == 0),
                            stop=(head_state["emitted"] == head_state["total"] - 1))
                        head_state["emitted"] += 1

            for l in range(3):
                for g0 in range(0, nslot, GEMM_GRP):
                    gn = min(GEMM_GRP, nslot - g0)
                    psum_t = pt_pool.tile([P, GEMM_GRP, HID], f32, tag="pt")
                    if l == 0:
                        xt = xpool.tile([FIN, GEMM_GRP * P], f32, tag="xt")
                        nc.sync.dma_start(
                            out=xt[:, :gn * P],
                            in_=x_dram[:, g0 * P:(g0 + gn) * P])
                    for j in range(gn):
                        if l == 0:
                            lhsT = xt[:, j * P:(j + 1) * P]
                        else:
                            lhsT = h_fm[:, (g0 + j) * P:(g0 + j + 1) * P]
                        nc.tensor.matmul(psum_t[:, j, :], lhsT, Wc_sb[l][:],
                                         start=True, stop=True)
                    nc.vector.tensor_mul(m_sb[:, g0:g0 + gn, :],
                                         psum_t[:, :gn, :],
                                         dinv64[:, g0:g0 + gn, :])
                    nc.sync.dma_start(
                        out=m_dram[l][:].rearrange("(p g) f -> p g f",
                                                   p=P)[:, g0:g0 + gn, :],
                        in_=m_sb[:, g0:g0 + gn, :])
                nc.vector.tensor_copy(s_sb[:], m_sb[:])
                # segment the post-gather tail: segment [b0,b1) of node blocks
                # is complete after the last gather chunk whose adds touch it;
                # emit its mult/transpose/relu/matvec right there so it
                # overlaps the remaining gather stream.
                seg_bounds = []
                b0 = 0
                for sz in ([4] * 4 + [8] * 2 + [16] * 16):
                    if b0 >= nslot:
                        break
                    seg_bounds.append((b0, min(b0 + sz, nslot)))
                    b0 += sz
                segs = []
                for (b0, b1) in seg_bounds:
                    last = 0
                    for ci, adds in enumerate(chunk_adds):
                        if any(sb < b1 and sb + nb > b0 for (_, sb, nb) in adds):
                            last = ci
                    segs.append((b0, b1, last))
                psum_g = pg_pool.tile([P, nslot], f32, tag="pg")

                def emit_seg_tail(b0, b1):
                    nc.vector.tensor_mul(s_sb[:, b0:b1, :],
                                         s_sb[:, b0:b1, :],
                                         dinv64[:, b0:b1, :])
                    for g0 in range(b0, b1, TR_GRP):
                        gn = min(TR_GRP, b1 - g0)
                        psum_tr = ptr_pool.tile([HID, TR_GRP, P], f32,
                                                tag="ptr")
                        for j in range(gn):
                            nc.tensor.transpose(psum_tr[:, j, :],
                                                s_sb[:, g0 + j, :], ident[:])
                        nc.scalar.activation(
                            h_fm[:, g0 * P:(g0 + gn) * P],
                            psum_tr[:, :gn, :].rearrange("f g p -> f (g p)"),
                            mybir.ActivationFunctionType.Relu,
                            bias=bc_sb[l][:], scale=1.0)
                    for j in range(b0, b1):
                        nc.tensor.matmul(psum_g[:, j:j + 1],
                                         h_fm[:, j * P:(j + 1) * P],
                                         wl_sb[l][:], start=True, stop=True)

                for ci, (cstart, clen) in enumerate(chunks):
                    cblk = clen // P
                    idx_t = ipool.tile([P, cfg.chunk // 16], mybir.dt.int16,
                                       tag="idx")
                    nc.sync.dma_start(
                        out=idx_t[:, :clen // 16],
                        in_=idx_dram[:, cstart // 16:(cstart + clen) // 16])
                    gbuf = gpool.tile([P, (cfg.chunk + P - 1) // P, HID], f32,
                                      tag="gb")
                    nc.gpsimd.dma_gather(
                        gbuf[:, :cblk, :], m_dram[l][:], idx_t[:, :clen // 16],
                        clen, clen, HID, single_packet=False)
                    for (gb, sb, nb) in chunk_adds[ci]:
                        nc.vector.tensor_add(s_sb[:, sb:sb + nb, :],
                                             s_sb[:, sb:sb + nb, :],
                                             gbuf[:, gb:gb + nb, :])
                    for si, (b0, b1, last) in enumerate(segs):
                        if last == ci:
                            emit_seg_tail(b0, b1)
                            if l == 2:
                                emit_head_seg(si, b0, b1)
                if l < 2:
                    nc.vector.tensor_add(g_acc[:], g_acc[:], psum_g[:])

            assert head_state["emitted"] == nslot, head_state
            z_sb = spool.tile([1, NFC], f32, tag="z")
            nc.vector.tensor_add(z_sb[:], psum_z[:], b1_sb[:])
            nc.vector.tensor_relu(z_sb[:], z_sb[:])
            nc.sync.dma_start(out=z_dram[:], in_=z_sb[:])

    nc.compile()
    return nc


def make_in_maps(cfg: Cfg, prep, consts, n_cores=N_CORES):
    eye = np.eye(P, dtype=np.float32)
    shared = dict(
        dinv64=consts["dinv64"], mask_nm=consts["mask_nm"],
        idx_all=prep["idx_wrapped"], W1T=consts["W1T"],
        b1_eff=consts["b1_eff"], ident=eye,
    )
    for i in range(3):
        shared[f"Wc{i}"] = consts["Wc"][i]
        shared[f"bc{i}"] = consts["bc"][i]
        shared[f"wl{i}"] = consts["w_l"][i]
    return [dict(shared, x_fm=np.ascontiguousarray(consts["x_fm"][c]))
            for c in range(n_cores)]


def finish_host(z_all, inputs):
    W2 = np.asarray(inputs["lin2_W"], np.float32)
    b2 = np.asarray(inputs["lin2_b"], np.float32)
    logits = z_all @ W2.T + b2
    mx = logits.max(axis=1, keepdims=True)
    e = np.exp(logits - mx)
    return ((logits - mx) - np.log(e.sum(axis=1, keepdims=True))).astype(np.float32)


_PROGRAM_CACHE = {}


def _get_program(cfg: Cfg, prep, cache_key):
    hit = _PROGRAM_CACHE.get(cache_key)
    if hit is None:
        hit = build_program(cfg, prep)
        _PROGRAM_CACHE[cache_key] = hit
    return hit


def _reset_device():
    """Run a trivial program to clear a wedged exec unit (observed to help)."""
    try:
        nc = bacc.Bacc("TRN2", debug=False)
        a = nc.dram_tensor("a", [P, 64], mybir.dt.float32, kind="ExternalInput")
        b = nc.dram_tensor("b", [P, 64], mybir.dt.float32, kind="ExternalOutput")
        with tile.TileContext(nc) as tc:
            with tc.tile_pool(name="p", bufs=1) as pool:
                t = pool.tile([P, 64], mybir.dt.float32)
                nc.sync.dma_start(out=t[:], in_=a[:])
                nc.sync.dma_start(out=b[:], in_=t[:])
        nc.compile()
        run_bass_kernel_spmd(
            nc, [{"a": np.zeros((P, 64), np.float32)}] * N_CORES,
            list(range(N_CORES)))
    except Exception:
        pass


def kernel(**inputs) -> np.ndarray:
    x = np.asarray(inputs["x"])
    bs, n = x.shape[0], x.shape[1]
    assert bs == N_CORES, f"expected batch {N_CORES}, got {bs}"

    cfg = Cfg(n=n)
    edge_index = np.asarray(inputs["edge_index"])
    prep = preprocess(cfg, edge_index)
    cache_key = (n, edge_index.shape[1],
                 hash(edge_index.tobytes()))
    nc = _get_program(cfg, prep, cache_key)
    consts = build_constants(cfg, prep, inputs)
    in_maps = make_in_maps(cfg, prep, consts)

    last_err = None
    for attempt in range(3):
        try:
            res = run_bass_kernel_spmd(nc, in_maps, list(range(N_CORES)))
            break
        except Exception as e:  # wedged device — reset and retry
            last_err = e
            _reset_device()
    else:
        raise last_err

    z_all = np.stack([res.results[c]["z"][0] for c in range(N_CORES)])
    return finish_host(z_all, inputs)



# revision 2
# speedup vs baseline: 1.7775x; 1.7775x over previous
"""TRN2 Bass kernel for nn_GCNEModel (3-layer GCN + dense head), 8 NeuronCores.

Sharding: data-parallel over the batch axis — each core runs one sample's
full GCN. The scatter-add aggregation is restructured as pure row-gathers:
nodes are relabeled by descending in-degree (the relabeling is absorbed into
host-side permutations of pel_W / x / lin1_W), so "round k" (the k-th
incoming edge of every node that has one) targets a contiguous node prefix.
Per layer, per core:

  t = x @ W^T            PE GEMM (stationary = feat-major input chunk)
  m = dinv * t           DVE, node-major [128, 119, 64]; DMA'd to HBM rows
  s = m + sum_k m[src_k] MoE-style dma_gather rounds + DVE adds
  h = relu(dinv*s^T + b) DVE mult, PE transpose, ACT bias+relu (feat-major)
  g += h^T @ w_l         PE matvec chunks (fc folded per layer)

head: z = relu(sum_j g_j^T @ lin1_W^T_j + b1); host: lin2 + log_softmax.
"""
import os
import sys

os.environ.setdefault("NEURON_RT_RESET_CORES", "1")
for _p in ("/opt/trn_rl_repo", "/root/.axon_site/_ro/trn_rl_repo"):
    if os.path.isdir(_p) and _p not in sys.path:
        sys.path.insert(0, _p)

from dataclasses import dataclass, field

import numpy as np

import concourse.bacc as bacc
import concourse.mybir as mybir
import concourse.tile as tile
from concourse.bass_utils import run_bass_kernel_spmd

P = 128
HID = 64
FIN = 36
NFC = 256
N_CORES = 8


@dataclass
class Cfg:
    n: int
    chunk: int = 3968   # gather chunk; <= ~4080 (SWDGE ring: 256 descs/engine)
    npad: int = field(init=False)
    nslot: int = field(init=False)

    def __post_init__(self):
        self.nslot = (self.n + P - 1) // P
        self.npad = self.nslot * P


def preprocess(cfg: Cfg, edge_index: np.ndarray):
    n, npad, nslot = cfg.n, cfg.npad, cfg.nslot
    src_old = np.asarray(edge_index[0], dtype=np.int64)
    dst_old = np.asarray(edge_index[1], dtype=np.int64)
    E = src_old.shape[0]

    deg = np.bincount(dst_old, minlength=n)
    pi = np.argsort(-deg, kind="stable")
    inv_pi = np.empty(n, dtype=np.int64)
    inv_pi[pi] = np.arange(n)
    deg_s = deg[pi]

    dinv_pad = np.zeros(npad)
    dinv_pad[:n] = 1.0 / np.sqrt(deg_s.astype(np.float64) + 1.0)

    src_new = inv_pi[src_old]
    dst_new = inv_pi[dst_old]

    order = np.argsort(dst_new, kind="stable")
    src_sorted = src_new[order]
    dst_sorted = dst_new[order]
    starts = np.zeros(n + 1, dtype=np.int64)
    np.cumsum(np.bincount(dst_new, minlength=n), out=starts[1:])
    kpos = np.arange(E) - starts[dst_sorted]

    Kmax = int(deg_s[0]) if E else 0
    DUMMY_SIG = n
    assert npad > n

    def r_of(sig):
        return (sig % P) * nslot + sig // P

    idx_stream = []
    segments = []
    stream_blk = 0
    for k in range(Kmax):
        n_k = int(np.searchsorted(-deg_s, -(k + 1), side="right"))
        sel = kpos == k
        srcs_k = src_sorted[sel][np.argsort(dst_sorted[sel], kind="stable")]
        assert srcs_k.shape[0] == n_k
        nblk = (n_k + P - 1) // P
        padded = np.full(nblk * P, DUMMY_SIG, dtype=np.int64)
        padded[:n_k] = srcs_k
        idx_stream.append(r_of(padded))
        segments.append((stream_blk, nblk))
        stream_blk += nblk

    idx_stream = (np.concatenate(idx_stream) if idx_stream
                  else np.zeros(0, np.int64))
    Eprime = idx_stream.shape[0]

    chunks = []
    pos = 0
    while pos < Eprime:
        c = min(cfg.chunk, Eprime - pos)
        chunks.append((pos, c))
        pos += c

    chunk_adds = []
    for (cstart, clen) in chunks:
        c_b0, c_b1 = cstart // P, (cstart + clen) // P
        adds = []
        for (sb, nb) in segments:
            lo, hi = max(c_b0, sb), min(c_b1, sb + nb)
            if lo < hi:
                adds.append((lo - c_b0, lo - sb, hi - lo))
        chunk_adds.append(adds)

    cols = Eprime // 16
    arr16 = idx_stream.reshape(cols, 16).T.astype(np.int16)
    idx_wrapped = np.ascontiguousarray(np.tile(arr16, (8, 1)))

    return dict(pi=pi, deg_s=deg_s, dinv_pad=dinv_pad,
                idx_stream=idx_stream, idx_wrapped=idx_wrapped,
                chunks=chunks, chunk_adds=chunk_adds, Eprime=Eprime)


def build_constants(cfg: Cfg, prep, inputs):
    n, npad, nslot = cfg.n, cfg.npad, cfg.nslot
    pi = prep["pi"]
    dinv_pad = prep["dinv_pad"].astype(np.float32)

    pel_W = np.asarray(inputs["pel_W"], np.float32)
    pel_b = np.asarray(inputs["pel_b"], np.float32)
    pe_perm = (pel_W.T + pel_b)[pi]

    x = np.asarray(inputs["x"], np.float32)
    bs = x.shape[0]
    x_fm = np.zeros((bs, FIN, npad), np.float32)
    for s in range(bs):
        xc = np.concatenate([x[s][pi], pe_perm], axis=1)
        x_fm[s, :, :n] = xc.T

    def to_node_major(v):
        return np.ascontiguousarray(v.reshape(nslot, P).T)

    dinv_nm = to_node_major(dinv_pad)
    dinv64 = np.ascontiguousarray(
        np.repeat(dinv_nm[:, :, None], HID, axis=2)).reshape(P, nslot * HID)
    mask = np.zeros(npad, np.float32)
    mask[:n] = 1.0
    mask_nm = to_node_major(mask)

    Wc = [np.ascontiguousarray(np.asarray(inputs[f"conv{i}_W"], np.float32).T)
          for i in (1, 2, 3)]
    bc = [np.ascontiguousarray(np.asarray(inputs[f"conv{i}_b"], np.float32)
                               .reshape(HID, 1)) for i in (1, 2, 3)]

    fc_W = np.asarray(inputs["fc_W"], np.float32).reshape(-1)
    w_l = [np.ascontiguousarray(fc_W[l::3].reshape(HID, 1)) for l in range(3)]
    fc_b = float(np.asarray(inputs["fc_b"], np.float32).reshape(()))

    import ml_dtypes
    lin1_W = np.asarray(inputs["lin1_W"], np.float32)
    W1T = np.zeros((npad, NFC), np.float32)
    W1T[:n] = lin1_W[:, pi].T
    W1T = np.ascontiguousarray(W1T.astype(ml_dtypes.bfloat16))
    b1_eff = np.ascontiguousarray(
        (np.asarray(inputs["lin1_b"], np.float32)
         + fc_b * lin1_W.sum(axis=1)).reshape(1, NFC))

    return dict(x_fm=x_fm, dinv64=dinv64, mask_nm=mask_nm, Wc=Wc, bc=bc,
                w_l=w_l, W1T=W1T, b1_eff=b1_eff)


def build_program(cfg: Cfg, prep, gb_bufs=2, idx_bufs=4, xw_bufs=2, w1_bufs=4, pt_bufs=3, ptr_bufs=2, gemm_grp=8, tr_grp=4):
    n, npad, nslot = cfg.n, cfg.npad, cfg.nslot
    chunks, chunk_adds = prep["chunks"], prep["chunk_adds"]
    cols_total = prep["Eprime"] // 16
    f32 = mybir.dt.float32

    nc = bacc.Bacc("TRN2", debug=False)

    x_dram = nc.dram_tensor("x_fm", [FIN, npad], f32, kind="ExternalInput")
    dinv_dram = nc.dram_tensor("dinv64", [P, nslot * HID], f32, kind="ExternalInput")
    mask_dram = nc.dram_tensor("mask_nm", [P, nslot], f32, kind="ExternalInput")
    Wc_dram = [nc.dram_tensor(f"Wc{i}", [FIN if i == 0 else HID, HID], f32,
                              kind="ExternalInput") for i in range(3)]
    bc_dram = [nc.dram_tensor(f"bc{i}", [HID, 1], f32, kind="ExternalInput")
               for i in range(3)]
    wl_dram = [nc.dram_tensor(f"wl{i}", [HID, 1], f32, kind="ExternalInput")
               for i in range(3)]
    idx_dram = nc.dram_tensor("idx_all", [P, cols_total], mybir.dt.int16,
                              kind="ExternalInput")
    w1t_dram = nc.dram_tensor("W1T", [npad, NFC], mybir.dt.bfloat16,
                              kind="ExternalInput")
    b1_dram = nc.dram_tensor("b1_eff", [1, NFC], f32, kind="ExternalInput")
    ident_dram = nc.dram_tensor("ident", [P, P], f32, kind="ExternalInput")
    z_dram = nc.dram_tensor("z", [1, NFC], f32, kind="ExternalOutput")

    m_dram = [nc.dram_tensor(f"m_hbm{i}", [npad, HID], f32) for i in range(3)]

    with tile.TileContext(nc) as tc:
        with (
            tc.tile_pool(name="const", bufs=1) as cpool,
            tc.tile_pool(name="state", bufs=1) as spool,
            tc.tile_pool(name="xw", bufs=xw_bufs) as xpool,
            tc.tile_pool(name="idx", bufs=idx_bufs) as ipool,
            tc.tile_pool(name="gath", bufs=gb_bufs) as gpool,
            tc.tile_pool(name="w1t", bufs=w1_bufs) as wpool,
            tc.tile_pool(name="psum_t", bufs=pt_bufs, space="PSUM") as pt_pool,
            tc.tile_pool(name="psum_tr", bufs=ptr_bufs, space="PSUM") as ptr_pool,
            tc.tile_pool(name="psum_g", bufs=1, space="PSUM") as pg_pool,
            tc.tile_pool(name="psum_z", bufs=1, space="PSUM") as pz_pool,
        ):
            dinv64 = cpool.tile([P, nslot, HID], f32, tag="dinv64")
            nc.sync.dma_start(out=dinv64[:], in_=dinv_dram[:].rearrange(
                "p (g f) -> p g f", f=HID))
            mask_sb = cpool.tile([P, nslot], f32, tag="mask")
            nc.sync.dma_start(out=mask_sb[:], in_=mask_dram[:])
            ident = cpool.tile([P, P], f32, tag="ident")
            nc.sync.dma_start(out=ident[:], in_=ident_dram[:])
            Wc_sb, bc_sb, wl_sb = [], [], []
            for i in range(3):
                w = cpool.tile([FIN if i == 0 else HID, HID], f32, tag=f"Wc{i}")
                nc.sync.dma_start(out=w[:], in_=Wc_dram[i][:])
                Wc_sb.append(w)
                b = cpool.tile([HID, 1], f32, tag=f"bc{i}")
                nc.sync.dma_start(out=b[:], in_=bc_dram[i][:])
                bc_sb.append(b)
                wl = cpool.tile([HID, 1], f32, tag=f"wl{i}")
                nc.sync.dma_start(out=wl[:], in_=wl_dram[i][:])
                wl_sb.append(wl)
            b1_sb = cpool.tile([1, NFC], f32, tag="b1")
            nc.sync.dma_start(out=b1_sb[:], in_=b1_dram[:])

            g_acc = spool.tile([P, nslot], f32, tag="g_acc")
            nc.vector.memset(g_acc[:], 0.0)
            m_sb = spool.tile([P, nslot, HID], f32, tag="m")
            s_sb = spool.tile([P, nslot, HID], f32, tag="s")
            h_fm = spool.tile([HID, npad], f32, tag="h_fm")

            GEMM_GRP = gemm_grp
            TR_GRP = tr_grp
            HGRP = 4
            psum_z = pz_pool.tile([1, NFC], f32, tag="pz")
            head_state = dict(emitted=0, total=nslot)

            def emit_head_seg(si, b0, b1):
                # finalize g for this segment and fold it into the lin1
                # matvec while layer-3 gathers still run
                nc.vector.tensor_add(g_acc[:, b0:b1], g_acc[:, b0:b1],
                                     psum_g[:, b0:b1])
                nc.vector.tensor_mul(g_acc[:, b0:b1], g_acc[:, b0:b1],
                                     mask_sb[:, b0:b1])
                g_bf = wpool.tile([P, b1 - b0], mybir.dt.bfloat16,
                                  tag="g_bf")
                nc.vector.tensor_copy(g_bf[:], g_acc[:, b0:b1])
                for g0 in range(b0, b1, HGRP):
                    gn = min(HGRP, b1 - g0)
                    w1t = wpool.tile([P, HGRP, NFC], mybir.dt.bfloat16,
                                     tag="w1t")
                    nc.sync.dma_start(
                        out=w1t[:, :gn, :],
                        in_=w1t_dram[:].rearrange(
                            "(g p) f -> p g f", p=P)[:, g0:g0 + gn, :])
                    for j in range(gn):
                        jj = g0 + j
                        nc.tensor.matmul(
                            psum_z[:], g_bf[:, jj - b0:jj - b0 + 1],
                            w1t[:, j, :],
                            start=(head_state["emitted"] == 0),
                            stop=(head_state["emitted"] == head_state["total"] - 1))
                        head_state["emitted"] += 1

            for l in range(3):
                for g0 in range(0, nslot, GEMM_GRP):
                    gn = min(GEMM_GRP, nslot - g0)
                    psum_t = pt_pool.tile([P, GEMM_GRP, HID], f32, tag="pt")
                    if l == 0:
                        xt = xpool.tile([FIN, GEMM_GRP * P], f32, tag="xt")
                        nc.sync.dma_start(
                            out=xt[:, :gn * P],
                            in_=x_dram[:, g0 * P:(g0 + gn) * P])
                    for j in range(gn):
                        if l == 0:
                            lhsT = xt[:, j * P:(j + 1) * P]
                        else:
                            lhsT = h_fm[:, (g0 + j) * P:(g0 + j + 1) * P]
                        nc.tensor.matmul(psum_t[:, j, :], lhsT, Wc_sb[l][:],
                                         start=True, stop=True)
                    nc.vector.tensor_mul(m_sb[:, g0:g0 + gn, :],
                                         psum_t[:, :gn, :],
                                         dinv64[:, g0:g0 + gn, :])
                    nc.sync.dma_start(
                        out=m_dram[l][:].rearrange("(p g) f -> p g f",
                                                   p=P)[:, g0:g0 + gn, :],
                        in_=m_sb[:, g0:g0 + gn, :])
                nc.vector.tensor_copy(s_sb[:], m_sb[:])
                # segment the post-gather tail: segment [b0,b1) of node blocks
                # is complete after the last gather chunk whose adds touch it;
                # emit its mult/transpose/relu/matvec right there so it
                # overlaps the remaining gather stream.
                seg_bounds = []
                b0 = 0
                for sz in ([4] * 4 + [8] * 2 + [16] * 16):
                    if b0 >= nslot:
                        break
                    seg_bounds.append((b0, min(b0 + sz, nslot)))
                    b0 += sz
                segs = []
                for (b0, b1) in seg_bounds:
                    last = 0
                    for ci, adds in enumerate(chunk_adds):
                        if any(sb < b1 and sb + nb > b0 for (_, sb, nb) in adds):
                            last = ci
                    segs.append((b0, b1, last))
                psum_g = pg_pool.tile([P, nslot], f32, tag="pg")

                def emit_seg_tail(b0, b1):
                    nc.vector.tensor_mul(s_sb[:, b0:b1, :],
                                         s_sb[:, b0:b1, :],
                                         dinv64[:, b0:b1, :])
                    for g0 in range(b0, b1, TR_GRP):
                        gn = min(TR_GRP, b1 - g0)
                        psum_tr = ptr_pool.tile([HID, TR_GRP, P], f32,
                                                tag="ptr")
                        for j in range(gn):
                            nc.tensor.transpose(psum_tr[:, j, :],
                                                s_sb[:, g0 + j, :], ident[:])
                        nc.scalar.activation(
                            h_fm[:, g0 * P:(g0 + gn) * P],
                            psum_tr[:, :gn, :].rearrange("f g p -> f (g p)"),
                            mybir.ActivationFunctionType.Relu,
                            bias=bc_sb[l][:], scale=1.0)
                    for j in range(b0, b1):
                        nc.tensor.matmul(psum_g[:, j:j + 1],
                                         h_fm[:, j * P:(j + 1) * P],
                                         wl_sb[l][:], start=True, stop=True)

                for ci, (cstart, clen) in enumerate(chunks):
                    cblk = clen // P
                    idx_t = ipool.tile([P, cfg.chunk // 16], mybir.dt.int16,
                                       tag="idx")
                    nc.sync.dma_start(
                        out=idx_t[:, :clen // 16],
                        in_=idx_dram[:, cstart // 16:(cstart + clen) // 16])
                    gbuf = gpool.tile([P, (cfg.chunk + P - 1) // P, HID], f32,
                                      tag="gb")
                    nc.gpsimd.dma_gather(
                        gbuf[:, :cblk, :], m_dram[l][:], idx_t[:, :clen // 16],
                        clen, clen, HID, single_packet=False)
                    for (gb, sb, nb) in chunk_adds[ci]:
                        nc.vector.tensor_add(s_sb[:, sb:sb + nb, :],
                                             s_sb[:, sb:sb + nb, :],
                                             gbuf[:, gb:gb + nb, :])
                    for si, (b0, b1, last) in enumerate(segs):
                        if last == ci:
                            emit_seg_tail(b0, b1)
                            if l == 2:
                                emit_head_seg(si, b0, b1)
                if l < 2:
                    nc.vector.tensor_add(g_acc[:], g_acc[:], psum_g[:])

            assert head_state["emitted"] == nslot, head_state
            z_sb = spool.tile([1, NFC], f32, tag="z")
            nc.vector.tensor_add(z_sb[:], psum_z[:], b1_sb[:])
            nc.vector.tensor_relu(z_sb[:], z_sb[:])
            nc.sync.dma_start(out=z_dram[:], in_=z_sb[:])

    nc.compile()
    return nc


def make_in_maps(cfg: Cfg, prep, consts, n_cores=N_CORES):
    eye = np.eye(P, dtype=np.float32)
    shared = dict(
        dinv64=consts["dinv64"], mask_nm=consts["mask_nm"],
        idx_all=prep["idx_wrapped"], W1T=consts["W1T"],
        b1_eff=consts["b1_eff"], ident=eye,
    )
    for i in range(3):
        shared[f"Wc{i}"] = consts["Wc"][i]
        shared[f"bc{i}"] = consts["bc"][i]
        shared[f"wl{i}"] = consts["w_l"][i]
    return [dict(shared, x_fm=np.ascontiguousarray(consts["x_fm"][c]))
            for c in range(n_cores)]


def finish_host(z_all, inputs):
    W2 = np.asarray(inputs["lin2_W"], np.float32)
    b2 = np.asarray(inputs["lin2_b"], np.float32)
    logits = z_all @ W2.T + b2
    mx = logits.max(axis=1, keepdims=True)
    e = np.exp(logits - mx)
    return ((logits - mx) - np.log(e.sum(axis=1, keepdims=True))).astype(np.float32)


_PROGRAM_CACHE = {}


def _get_program(cfg: Cfg, prep, cache_key):
    hit = _PROGRAM_CACHE.get(cache_key)
    if hit is None:
        hit = build_program(cfg, prep)
        _PROGRAM_CACHE[cache_key] = hit
    return hit


def _reset_device():
    """Run a trivial program to clear a wedged exec unit (observed to help)."""
    try:
        nc = bacc.Bacc("TRN2", debug=False)
        a = nc.dram_tensor("a", [P, 64], mybir.dt.float32, kind="ExternalInput")
        b = nc.dram_tensor("b", [P, 64], mybir.dt.float32, kind="ExternalOutput")
        with tile.TileContext(nc) as tc:
            with tc.tile_pool(name="p", bufs=1) as pool:
                t = pool.tile([P, 64], mybir.dt.float32)
                nc.sync.dma_start(out=t[:], in_=a[:])
                nc.sync.dma_start(out=b[:], in_=t[:])
        nc.compile()
        run_bass_kernel_spmd(
            nc, [{"a": np.zeros((P, 64), np.float32)}] * N_CORES,
            list(range(N_CORES)))
    except Exception:
        pass


def kernel(**inputs) -> np.ndarray:
    x = np.asarray(inputs["x"])
    bs, n = x.shape[0], x.shape[1]
    assert bs == N_CORES, f"expected batch {N_CORES}, got {bs}"

    cfg = Cfg(n=n)
    edge_index = np.asarray(inputs["edge_index"])
    prep = preprocess(cfg, edge_index)
    cache_key = (n, edge_index.shape[1],
                 hash(edge_index.tobytes()))
    nc = _get_program(cfg, prep, cache_key)
    consts = build_constants(cfg, prep, inputs)
    in_maps = make_in_maps(cfg, prep, consts)

    last_err = None
    for attempt in range(3):
        try:
            res = run_bass_kernel_spmd(nc, in_maps, list(range(N_CORES)))
            break
        except Exception as e:  # wedged device — reset and retry
            last_err = e
            _reset_device()
    else:
        raise last_err

    z_all = np.stack([res.results[c]["z"][0] for c in range(N_CORES)])
    return finish_host(z_all, inputs)



# revision 3
# speedup vs baseline: 1.7932x; 1.0089x over previous
"""TRN2 Bass kernel for nn_GCNEModel (3-layer GCN + dense head), 8 NeuronCores.

Sharding: data-parallel over the batch axis — each core runs one sample's
full GCN. The scatter-add aggregation is restructured as pure row-gathers:
nodes are relabeled by descending in-degree (the relabeling is absorbed into
host-side permutations of pel_W / x / lin1_W), so "round k" (the k-th
incoming edge of every node that has one) targets a contiguous node prefix.
Per layer, per core:

  t = x @ W^T            PE GEMM (stationary = feat-major input chunk)
  m = dinv * t           DVE, node-major [128, 119, 64]; DMA'd to HBM rows
  s = m + sum_k m[src_k] MoE-style dma_gather rounds + DVE adds
  h = relu(dinv*s^T + b) DVE mult, PE transpose, ACT bias+relu (feat-major)
  g += h^T @ w_l         PE matvec chunks (fc folded per layer)

head: z = relu(sum_j g_j^T @ lin1_W^T_j + b1); host: lin2 + log_softmax.
"""
import os
import sys

os.environ.setdefault("NEURON_RT_RESET_CORES", "1")
for _p in ("/opt/trn_rl_repo", "/root/.axon_site/_ro/trn_rl_repo"):
    if os.path.isdir(_p) and _p not in sys.path:
        sys.path.insert(0, _p)

from dataclasses import dataclass, field

import numpy as np

import concourse.bacc as bacc
import concourse.mybir as mybir
import concourse.tile as tile
from concourse.bass_utils import run_bass_kernel_spmd

P = 128
HID = 64
FIN = 36
NFC = 256
N_CORES = 8


@dataclass
class Cfg:
    n: int
    chunk: int = 3968   # gather chunk; <= ~4080 (SWDGE ring: 256 descs/engine)
    npad: int = field(init=False)
    nslot: int = field(init=False)

    def __post_init__(self):
        self.nslot = (self.n + P - 1) // P
        self.npad = self.nslot * P


def preprocess(cfg: Cfg, edge_index: np.ndarray):
    n, npad, nslot = cfg.n, cfg.npad, cfg.nslot
    src_old = np.asarray(edge_index[0], dtype=np.int64)
    dst_old = np.asarray(edge_index[1], dtype=np.int64)
    E = src_old.shape[0]

    deg = np.bincount(dst_old, minlength=n)
    pi = np.argsort(-deg, kind="stable")
    inv_pi = np.empty(n, dtype=np.int64)
    inv_pi[pi] = np.arange(n)
    deg_s = deg[pi]

    dinv_pad = np.zeros(npad)
    dinv_pad[:n] = 1.0 / np.sqrt(deg_s.astype(np.float64) + 1.0)

    src_new = inv_pi[src_old]
    dst_new = inv_pi[dst_old]

    order = np.argsort(dst_new, kind="stable")
    src_sorted = src_new[order]
    dst_sorted = dst_new[order]
    starts = np.zeros(n + 1, dtype=np.int64)
    np.cumsum(np.bincount(dst_new, minlength=n), out=starts[1:])
    kpos = np.arange(E) - starts[dst_sorted]

    Kmax = int(deg_s[0]) if E else 0
    DUMMY_SIG = n
    assert npad > n

    def r_of(sig):
        return (sig % P) * nslot + sig // P

    idx_stream = []
    segments = []
    stream_blk = 0
    for k in range(Kmax):
        n_k = int(np.searchsorted(-deg_s, -(k + 1), side="right"))
        sel = kpos == k
        srcs_k = src_sorted[sel][np.argsort(dst_sorted[sel], kind="stable")]
        assert srcs_k.shape[0] == n_k
        nblk = (n_k + P - 1) // P
        padded = np.full(nblk * P, DUMMY_SIG, dtype=np.int64)
        padded[:n_k] = srcs_k
        idx_stream.append(r_of(padded))
        segments.append((stream_blk, nblk))
        stream_blk += nblk

    idx_stream = (np.concatenate(idx_stream) if idx_stream
                  else np.zeros(0, np.int64))
    Eprime = idx_stream.shape[0]

    chunks = []
    pos = 0
    while pos < Eprime:
        c = min(cfg.chunk, Eprime - pos)
        chunks.append((pos, c))
        pos += c

    chunk_adds = []
    for (cstart, clen) in chunks:
        c_b0, c_b1 = cstart // P, (cstart + clen) // P
        adds = []
        for (sb, nb) in segments:
            lo, hi = max(c_b0, sb), min(c_b1, sb + nb)
            if lo < hi:
                adds.append((lo - c_b0, lo - sb, hi - lo))
        chunk_adds.append(adds)

    # pack idx per chunk, each padded to 256 columns (512B rows) so the
    # idx-load DMA descriptors avoid the <512B small-transfer penalty
    CCOLS = 256
    blocks = []
    for (cstart, clen) in chunks:
        a = idx_stream[cstart:cstart + clen].reshape(-1, 16).T.astype(np.int16)
        pad = np.zeros((16, CCOLS - a.shape[1]), np.int16)
        blocks.append(np.concatenate([a, pad], axis=1))
    arr16 = (np.concatenate(blocks, axis=1) if blocks
             else np.zeros((16, CCOLS), np.int16))
    idx_wrapped = np.ascontiguousarray(np.tile(arr16, (8, 1)))

    return dict(pi=pi, deg_s=deg_s, dinv_pad=dinv_pad,
                idx_stream=idx_stream, idx_wrapped=idx_wrapped,
                chunks=chunks, chunk_adds=chunk_adds, Eprime=Eprime)


def build_constants(cfg: Cfg, prep, inputs):
    n, npad, nslot = cfg.n, cfg.npad, cfg.nslot
    pi = prep["pi"]
    dinv_pad = prep["dinv_pad"].astype(np.float32)

    pel_W = np.asarray(inputs["pel_W"], np.float32)
    pel_b = np.asarray(inputs["pel_b"], np.float32)
    pe_perm = (pel_W.T + pel_b)[pi]

    x = np.asarray(inputs["x"], np.float32)
    bs = x.shape[0]
    x_fm = np.zeros((bs, FIN, npad), np.float32)
    for s in range(bs):
        xc = np.concatenate([x[s][pi], pe_perm], axis=1)
        x_fm[s, :, :n] = xc.T

    def to_node_major(v):
        return np.ascontiguousarray(v.reshape(nslot, P).T)

    dinv_nm = to_node_major(dinv_pad)
    dinv64 = np.ascontiguousarray(
        np.repeat(dinv_nm[:, :, None], HID, axis=2)).reshape(P, nslot * HID)
    mask = np.zeros(npad, np.float32)
    mask[:n] = 1.0
    mask_nm = to_node_major(mask)

    Wc = [np.ascontiguousarray(np.asarray(inputs[f"conv{i}_W"], np.float32).T)
          for i in (1, 2, 3)]
    bc = [np.ascontiguousarray(np.asarray(inputs[f"conv{i}_b"], np.float32)
                               .reshape(HID, 1)) for i in (1, 2, 3)]

    fc_W = np.asarray(inputs["fc_W"], np.float32).reshape(-1)
    w_l = [np.ascontiguousarray(fc_W[l::3].reshape(HID, 1)) for l in range(3)]
    fc_b = float(np.asarray(inputs["fc_b"], np.float32).reshape(()))

    import ml_dtypes
    lin1_W = np.asarray(inputs["lin1_W"], np.float32)
    W1T = np.zeros((npad, NFC), np.float32)
    W1T[:n] = lin1_W[:, pi].T
    W1T = np.ascontiguousarray(W1T.astype(ml_dtypes.bfloat16))
    b1_eff = np.ascontiguousarray(
        (np.asarray(inputs["lin1_b"], np.float32)
         + fc_b * lin1_W.sum(axis=1)).reshape(1, NFC))

    return dict(x_fm=x_fm, dinv64=dinv64, mask_nm=mask_nm, Wc=Wc, bc=bc,
                w_l=w_l, W1T=W1T, b1_eff=b1_eff)


def build_program(cfg: Cfg, prep, gb_bufs=2, idx_bufs=4, xw_bufs=2, w1_bufs=4, pt_bufs=3, ptr_bufs=2, gemm_grp=8, tr_grp=4):
    n, npad, nslot = cfg.n, cfg.npad, cfg.nslot
    chunks, chunk_adds = prep["chunks"], prep["chunk_adds"]
    cols_total = prep["Eprime"] // 16
    f32 = mybir.dt.float32

    nc = bacc.Bacc("TRN2", debug=False)

    x_dram = nc.dram_tensor("x_fm", [FIN, npad], f32, kind="ExternalInput")
    dinv_dram = nc.dram_tensor("dinv64", [P, nslot * HID], f32, kind="ExternalInput")
    mask_dram = nc.dram_tensor("mask_nm", [P, nslot], f32, kind="ExternalInput")
    Wc_dram = [nc.dram_tensor(f"Wc{i}", [FIN if i == 0 else HID, HID], f32,
                              kind="ExternalInput") for i in range(3)]
    bc_dram = [nc.dram_tensor(f"bc{i}", [HID, 1], f32, kind="ExternalInput")
               for i in range(3)]
    wl_dram = [nc.dram_tensor(f"wl{i}", [HID, 1], f32, kind="ExternalInput")
               for i in range(3)]
    idx_dram = nc.dram_tensor("idx_all", [P, 256 * len(chunks)],
                              mybir.dt.int16, kind="ExternalInput")
    w1t_dram = nc.dram_tensor("W1T", [npad, NFC], mybir.dt.bfloat16,
                              kind="ExternalInput")
    b1_dram = nc.dram_tensor("b1_eff", [1, NFC], f32, kind="ExternalInput")
    ident_dram = nc.dram_tensor("ident", [P, P], f32, kind="ExternalInput")
    z_dram = nc.dram_tensor("z", [1, NFC], f32, kind="ExternalOutput")

    m_dram = [nc.dram_tensor(f"m_hbm{i}", [npad, HID], f32) for i in range(3)]

    with tile.TileContext(nc) as tc:
        with (
            tc.tile_pool(name="const", bufs=1) as cpool,
            tc.tile_pool(name="state", bufs=1) as spool,
            tc.tile_pool(name="xw", bufs=xw_bufs) as xpool,
            tc.tile_pool(name="idx", bufs=idx_bufs) as ipool,
            tc.tile_pool(name="gath", bufs=gb_bufs) as gpool,
            tc.tile_pool(name="w1t", bufs=w1_bufs) as wpool,
            tc.tile_pool(name="psum_t", bufs=pt_bufs, space="PSUM") as pt_pool,
            tc.tile_pool(name="psum_tr", bufs=ptr_bufs, space="PSUM") as ptr_pool,
            tc.tile_pool(name="psum_g", bufs=1, space="PSUM") as pg_pool,
            tc.tile_pool(name="psum_z", bufs=1, space="PSUM") as pz_pool,
        ):
            dinv64 = cpool.tile([P, nslot, HID], f32, tag="dinv64")
            nc.sync.dma_start(out=dinv64[:], in_=dinv_dram[:].rearrange(
                "p (g f) -> p g f", f=HID))
            mask_sb = cpool.tile([P, nslot], f32, tag="mask")
            nc.sync.dma_start(out=mask_sb[:], in_=mask_dram[:])
            ident = cpool.tile([P, P], f32, tag="ident")
            nc.sync.dma_start(out=ident[:], in_=ident_dram[:])
            Wc_sb, bc_sb, wl_sb = [], [], []
            for i in range(3):
                w = cpool.tile([FIN if i == 0 else HID, HID], f32, tag=f"Wc{i}")
                nc.sync.dma_start(out=w[:], in_=Wc_dram[i][:])
                Wc_sb.append(w)
                b = cpool.tile([HID, 1], f32, tag=f"bc{i}")
                nc.sync.dma_start(out=b[:], in_=bc_dram[i][:])
                bc_sb.append(b)
                wl = cpool.tile([HID, 1], f32, tag=f"wl{i}")
                nc.sync.dma_start(out=wl[:], in_=wl_dram[i][:])
                wl_sb.append(wl)
            b1_sb = cpool.tile([1, NFC], f32, tag="b1")
            nc.sync.dma_start(out=b1_sb[:], in_=b1_dram[:])

            g_acc = spool.tile([P, nslot], f32, tag="g_acc")
            nc.vector.memset(g_acc[:], 0.0)
            m_sb = spool.tile([P, nslot, HID], f32, tag="m")
            s_sb = spool.tile([P, nslot, HID], f32, tag="s")
            h_fm = spool.tile([HID, npad], f32, tag="h_fm")

            GEMM_GRP = gemm_grp
            TR_GRP = tr_grp
            HGRP = 4
            psum_z = pz_pool.tile([1, NFC], f32, tag="pz")
            head_state = dict(emitted=0, total=nslot)

            def emit_head_seg(si, b0, b1):
                # finalize g for this segment and fold it into the lin1
                # matvec while layer-3 gathers still run
                nc.vector.tensor_add(g_acc[:, b0:b1], g_acc[:, b0:b1],
                                     psum_g[:, b0:b1])
                nc.vector.tensor_mul(g_acc[:, b0:b1], g_acc[:, b0:b1],
                                     mask_sb[:, b0:b1])
                g_bf = wpool.tile([P, b1 - b0], mybir.dt.bfloat16,
                                  tag="g_bf")
                nc.vector.tensor_copy(g_bf[:], g_acc[:, b0:b1])
                for g0 in range(b0, b1, HGRP):
                    gn = min(HGRP, b1 - g0)
                    w1t = wpool.tile([P, HGRP, NFC], mybir.dt.bfloat16,
                                     tag="w1t")
                    nc.sync.dma_start(
                        out=w1t[:, :gn, :],
                        in_=w1t_dram[:].rearrange(
                            "(g p) f -> p g f", p=P)[:, g0:g0 + gn, :])
                    for j in range(gn):
                        jj = g0 + j
                        nc.tensor.matmul(
                            psum_z[:], g_bf[:, jj - b0:jj - b0 + 1],
                            w1t[:, j, :],
                            start=(head_state["emitted"] == 0),
                            stop=(head_state["emitted"] == head_state["total"] - 1))
                        head_state["emitted"] += 1

            for l in range(3):
                for g0 in range(0, nslot, GEMM_GRP):
                    gn = min(GEMM_GRP, nslot - g0)
                    psum_t = pt_pool.tile([P, GEMM_GRP, HID], f32, tag="pt")
                    if l == 0:
                        xt = xpool.tile([FIN, GEMM_GRP * P], f32, tag="xt")
                        nc.sync.dma_start(
                            out=xt[:, :gn * P],
                            in_=x_dram[:, g0 * P:(g0 + gn) * P])
                    for j in range(gn):
                        if l == 0:
                            lhsT = xt[:, j * P:(j + 1) * P]
                        else:
                            lhsT = h_fm[:, (g0 + j) * P:(g0 + j + 1) * P]
                        nc.tensor.matmul(psum_t[:, j, :], lhsT, Wc_sb[l][:],
                                         start=True, stop=True)
                    nc.vector.tensor_mul(m_sb[:, g0:g0 + gn, :],
                                         psum_t[:, :gn, :],
                                         dinv64[:, g0:g0 + gn, :])
                    nc.sync.dma_start(
                        out=m_dram[l][:].rearrange("(p g) f -> p g f",
                                                   p=P)[:, g0:g0 + gn, :],
                        in_=m_sb[:, g0:g0 + gn, :])
                nc.vector.tensor_copy(s_sb[:], m_sb[:])
                # segment the post-gather tail: segment [b0,b1) of node blocks
                # is complete after the last gather chunk whose adds touch it;
                # emit its mult/transpose/relu/matvec right there so it
                # overlaps the remaining gather stream.
                seg_bounds = []
                b0 = 0
                for sz in ([4] * 4 + [8] * 2 + [16] * 16):
                    if b0 >= nslot:
                        break
                    seg_bounds.append((b0, min(b0 + sz, nslot)))
                    b0 += sz
                segs = []
                for (b0, b1) in seg_bounds:
                    last = 0
                    for ci, adds in enumerate(chunk_adds):
                        if any(sb < b1 and sb + nb > b0 for (_, sb, nb) in adds):
                            last = ci
                    segs.append((b0, b1, last))
                psum_g = pg_pool.tile([P, nslot], f32, tag="pg")

                def emit_seg_tail(b0, b1):
                    nc.vector.tensor_mul(s_sb[:, b0:b1, :],
                                         s_sb[:, b0:b1, :],
                                         dinv64[:, b0:b1, :])
                    for g0 in range(b0, b1, TR_GRP):
                        gn = min(TR_GRP, b1 - g0)
                        psum_tr = ptr_pool.tile([HID, TR_GRP, P], f32,
                                                tag="ptr")
                        for j in range(gn):
                            nc.tensor.transpose(psum_tr[:, j, :],
                                                s_sb[:, g0 + j, :], ident[:])
                        nc.scalar.activation(
                            h_fm[:, g0 * P:(g0 + gn) * P],
                            psum_tr[:, :gn, :].rearrange("f g p -> f (g p)"),
                            mybir.ActivationFunctionType.Relu,
                            bias=bc_sb[l][:], scale=1.0)
                    for j in range(b0, b1):
                        nc.tensor.matmul(psum_g[:, j:j + 1],
                                         h_fm[:, j * P:(j + 1) * P],
                                         wl_sb[l][:], start=True, stop=True)

                for ci, (cstart, clen) in enumerate(chunks):
                    cblk = clen // P
                    idx_t = ipool.tile([P, 256], mybir.dt.int16, tag="idx")
                    nc.sync.dma_start(
                        out=idx_t[:],
                        in_=idx_dram[:, ci * 256:(ci + 1) * 256])
                    gbuf = gpool.tile([P, (cfg.chunk + P - 1) // P, HID], f32,
                                      tag="gb")
                    nc.gpsimd.dma_gather(
                        gbuf[:, :cblk, :], m_dram[l][:], idx_t[:, :clen // 16],
                        clen, clen, HID, single_packet=False)
                    for (gb, sb, nb) in chunk_adds[ci]:
                        nc.vector.tensor_add(s_sb[:, sb:sb + nb, :],
                                             s_sb[:, sb:sb + nb, :],
                                             gbuf[:, gb:gb + nb, :])
                    for si, (b0, b1, last) in enumerate(segs):
                        if last == ci:
                            emit_seg_tail(b0, b1)
                            if l == 2:
                                emit_head_seg(si, b0, b1)
                if l < 2:
                    nc.vector.tensor_add(g_acc[:], g_acc[:], psum_g[:])

            assert head_state["emitted"] == nslot, head_state
            z_sb = spool.tile([1, NFC], f32, tag="z")
            nc.vector.tensor_add(z_sb[:], psum_z[:], b1_sb[:])
            nc.vector.tensor_relu(z_sb[:], z_sb[:])
            nc.sync.dma_start(out=z_dram[:], in_=z_sb[:])

    nc.compile()
    return nc


def make_in_maps(cfg: Cfg, prep, consts, n_cores=N_CORES):
    eye = np.eye(P, dtype=np.float32)
    shared = dict(
        dinv64=consts["dinv64"], mask_nm=consts["mask_nm"],
        idx_all=prep["idx_wrapped"], W1T=consts["W1T"],
        b1_eff=consts["b1_eff"], ident=eye,
    )
    for i in range(3):
        shared[f"Wc{i}"] = consts["Wc"][i]
        shared[f"bc{i}"] = consts["bc"][i]
        shared[f"wl{i}"] = consts["w_l"][i]
    return [dict(shared, x_fm=np.ascontiguousarray(consts["x_fm"][c]))
            for c in range(n_cores)]


def finish_host(z_all, inputs):
    W2 = np.asarray(inputs["lin2_W"], np.float32)
    b2 = np.asarray(inputs["lin2_b"], np.float32)
    logits = z_all @ W2.T + b2
    mx = logits.max(axis=1, keepdims=True)
    e = np.exp(logits - mx)
    return ((logits - mx) - np.log(e.sum(axis=1, keepdims=True))).astype(np.float32)


_PROGRAM_CACHE = {}


def _get_program(cfg: Cfg, prep, cache_key):
    hit = _PROGRAM_CACHE.get(cache_key)
    if hit is None:
        hit = build_program(cfg, prep)
        _PROGRAM_CACHE[cache_key] = hit
    return hit


def _reset_device():
    """Run a trivial program to clear a wedged exec unit (observed to help)."""
    try:
        nc = bacc.Bacc("TRN2", debug=False)
        a = nc.dram_tensor("a", [P, 64], mybir.dt.float32, kind="ExternalInput")
        b = nc.dram_tensor("b", [P, 64], mybir.dt.float32, kind="ExternalOutput")
        with tile.TileContext(nc) as tc:
            with tc.tile_pool(name="p", bufs=1) as pool:
                t = pool.tile([P, 64], mybir.dt.float32)
                nc.sync.dma_start(out=t[:], in_=a[:])
                nc.sync.dma_start(out=b[:], in_=t[:])
        nc.compile()
        run_bass_kernel_spmd(
            nc, [{"a": np.zeros((P, 64), np.float32)}] * N_CORES,
            list(range(N_CORES)))
    except Exception:
        pass


def kernel(**inputs) -> np.ndarray:
    x = np.asarray(inputs["x"])
    bs, n = x.shape[0], x.shape[1]
    assert bs == N_CORES, f"expected batch {N_CORES}, got {bs}"

    cfg = Cfg(n=n)
    edge_index = np.asarray(inputs["edge_index"])
    prep = preprocess(cfg, edge_index)
    cache_key = (n, edge_index.shape[1],
                 hash(edge_index.tobytes()))
    nc = _get_program(cfg, prep, cache_key)
    consts = build_constants(cfg, prep, inputs)
    in_maps = make_in_maps(cfg, prep, consts)

    last_err = None
    for attempt in range(3):
        try:
            res = run_bass_kernel_spmd(nc, in_maps, list(range(N_CORES)))
            break
        except Exception as e:  # wedged device — reset and retry
            last_err = e
            _reset_device()
    else:
        raise last_err

    z_all = np.stack([res.results[c]["z"][0] for c in range(N_CORES)])
    return finish_host(z_all, inputs)

